# revision 12
# baseline (speedup 1.0000x reference)
"""Trainium2 Bass kernel for nn_MessageFunction (gnn_message_passing).

Math (validated against the reference):
  The reference broadcasts h_w[:, :, None] -> (B*N, IN_F, N) and reshapes to
  [E, IN_F]; row-major order makes every row constant:
      h_w_rows[e, i] = h_w.reshape(-1)[e]   for all i.
  Hence the per-edge bmm collapses:
      m[e, o] = sum_i edge_output[e, o, i] * s[e]
              = s[e] * (x3[e] @ W4s[:, o] + b4s[o])
  with W4s = W4.reshape(HID3, OUT_F, IN_F).sum(-1), b4s = b4.reshape(OUT_F,
  IN_F).sum(-1), s = h_w.reshape(-1).  This is an exact reassociation (only
  f32 rounding differences) and removes the [E,128]@[128,4096] matmul + bmm.

Kernel: data-parallel over E = 32768 edges, 4096 per core across 8 cores,
MLP weights replicated, no cross-core communication.  Per core the MLP runs
features-on-partitions with edges streaming on the free dim:
    x1 = relu(W1.T @ eT)        K=32  -> [128, e]
    x2 = relu(W2.T @ x1)        K=128 -> [256, e] (two 128-part halves)
    x3 = relu(W3.T @ x2)        K=256 -> [128, e] (PSUM accumulation)
    y  = W4s.T @ x3             K=128 -> [64, e]  (col-packed 2 tiles/PSUM)
    out = (y + b4s) * s         one fused scalar_tensor_tensor on VectorE
Matmuls use float32r (full PE rate at N=512, near-fp32 precision).
"""

import os

import numpy as np

import concourse.bacc as bacc
import concourse.bass as bass
import concourse.mybir as mybir
import concourse.tile as tile
from concourse.bass_utils import run_bass_kernel_spmd

# Problem constants (hardcoded per the harness contract).
B, N = 8, 64
IN_F, OUT_F = 64, 64
EDGE_F = 32
HID1, HID2, HID3 = 128, 256, 128
E = B * N * N            # 32768
N_CORES = 8
E_LOC = E // N_CORES     # 4096
TILE = 1024              # edges per tile
NT = E_LOC // TILE       # 4 tiles per core

F32 = mybir.dt.float32
# Matmul operand dtype: float32r streams at 1 cycle/row for N>=256 (same as
# bf16) with much better precision than bf16.
DT = mybir.dt.float32r
NP_DT = np.float32

# Module global: last BassKernelResults (test.py reads exec_time_ns from it).
LAST_RESULTS = None


def _build_bass():
    nc = bacc.Bacc(
        "TRN2", target_bir_lowering=False, debug=False, num_devices=N_CORES
    )

    # Per-core inputs
    e_t = nc.dram_tensor("e_t", [EDGE_F, E_LOC], DT, kind="ExternalInput")
    s_b = nc.dram_tensor("s_b", [NT, OUT_F, TILE], F32, kind="ExternalInput")
    # Replicated weights
    w1d = nc.dram_tensor("w1d", [EDGE_F, HID1], DT, kind="ExternalInput")
    w2d = nc.dram_tensor("w2d", [HID1, HID2], DT, kind="ExternalInput")
    # W3 packed side by side: [:, 0:128] = W3[0:128, :], [:, 128:256] = W3[128:256, :]
    w3d = nc.dram_tensor("w3d", [128, 2 * HID3], DT, kind="ExternalInput")
    w4d = nc.dram_tensor("w4d", [HID3, OUT_F], DT, kind="ExternalInput")
    # Bias columns: b1, b2[:128], b2[128:], b3, [b4s; pad]
    bbd = nc.dram_tensor("bbd", [128, 5], F32, kind="ExternalInput")
    outd = nc.dram_tensor("outd", [NT, OUT_F, TILE], F32, kind="ExternalOutput")

    # Relu pass engine schedule (per tile: L1, L2a, L2b, L3). 'A' = ScalarE,
    # 'V' = VectorE.  VectorE also runs the four final bias+scale ops, so
    # ScalarE takes more of the 16 relu passes (10 A / 6 V).
    relu_sched = [
        "AVAV",
        "AVAA",
        "AVAV",
        "AVAA",
    ]

    with tile.TileContext(nc) as tc:
        with (
            tc.tile_pool(name="wp", bufs=1) as wp,
            tc.tile_pool(name="io", bufs=3) as io,
            tc.tile_pool(name="acts", bufs=2) as acts,
            tc.tile_pool(name="x3p", bufs=3) as x3pool,
            tc.tile_pool(name="op", bufs=2) as op,
            tc.tile_pool(name="ps", bufs=4, space="PSUM") as ps,
        ):
            w1 = wp.tile([EDGE_F, HID1], DT, tag="w1")
            w2 = wp.tile([HID1, HID2], DT, tag="w2")
            w3 = wp.tile([128, 2 * HID3], DT, tag="w3")
            w4 = wp.tile([HID3, OUT_F], DT, tag="w4")
            bb = wp.tile([128, 5], F32, tag="bb")
            nc.sync.dma_start(w1[:], w1d[:])
            nc.sync.dma_start(w2[:], w2d[:])
            nc.sync.dma_start(w3[:], w3d[:])
            nc.sync.dma_start(w4[:], w4d[:])
            nc.sync.dma_start(bb[:], bbd[:])

            def relu_pass(dst, src, bias_col, eng):
                if eng == "A":
                    nc.scalar.activation(
                        dst, src, mybir.ActivationFunctionType.Relu, bias=bias_col
                    )
                else:
                    nc.vector.tensor_scalar(
                        out=dst,
                        in0=src,
                        scalar1=bias_col,
                        scalar2=0.0,
                        op0=mybir.AluOpType.add,
                        op1=mybir.AluOpType.max,
                    )

            for t in range(NT):
                sched = relu_sched[t]

                s_t = io.tile([OUT_F, TILE], F32, tag="s_t")
                nc.sync.dma_start(s_t[:], s_b[t])

                et = io.tile([EDGE_F, TILE], DT, tag="et")
                nc.sync.dma_start(et[:], e_t[:, t * TILE : (t + 1) * TILE])

                # L1: [32,128].T @ [32,1024]
                x1p = ps.tile([128, TILE], F32, tag="ps")
                for h in range(2):
                    hs = slice(h * 512, (h + 1) * 512)
                    nc.tensor.matmul(x1p[:, hs], w1[:], et[:, hs])
                x1 = acts.tile([128, TILE], DT, tag="x1")
                relu_pass(x1[:], x1p[:], bb[:, 0:1], sched[0])

                # L2: [128,256] -> two 128-part halves
                x2ap = ps.tile([128, TILE], F32, tag="ps")
                x2bp = ps.tile([128, TILE], F32, tag="ps")
                for h in range(2):
                    hs = slice(h * 512, (h + 1) * 512)
                    nc.tensor.matmul(x2ap[:, hs], w2[:, 0:128], x1[:, hs])
                    nc.tensor.matmul(x2bp[:, hs], w2[:, 128:256], x1[:, hs])
                x2a = acts.tile([128, TILE], DT, tag="x2a")
                x2b = acts.tile([128, TILE], DT, tag="x2b")
                relu_pass(x2a[:], x2ap[:], bb[:, 1:2], sched[1])
                relu_pass(x2b[:], x2bp[:], bb[:, 2:3], sched[2])

                # L3: K=256 via PSUM accumulation of two K=128 matmuls
                x3ps = ps.tile([128, TILE], F32, tag="ps")
                for h in range(2):
                    hs = slice(h * 512, (h + 1) * 512)
                    nc.tensor.matmul(
                        x3ps[:, hs], w3[:, 0:128], x2a[:, hs],
                        start=True, stop=False,
                    )
                    nc.tensor.matmul(
                        x3ps[:, hs], w3[:, 128:256], x2b[:, hs],
                        start=False, stop=True,
                    )
                x3 = x3pool.tile([128, TILE], DT, tag="x3")
                relu_pass(x3[:], x3ps[:], bb[:, 3:4], sched[3])

                # L4: [128,64].T @ [128,1024] -> [64, 1024]
                y4p = ps.tile([OUT_F, TILE], F32, tag="ps")
                for h in range(2):
                    hs = slice(h * 512, (h + 1) * 512)
                    nc.tensor.matmul(y4p[:, hs], w4[:], x3[:, hs])
                # out = (y4 + b4s) * s   (single fused VectorE op)
                ob = op.tile([OUT_F, TILE], F32, tag="ob")
                nc.vector.scalar_tensor_tensor(
                    out=ob[:],
                    in0=y4p[:],
                    scalar=bb[0:OUT_F, 4:5],
                    in1=s_t[:],
                    op0=mybir.AluOpType.add,
                    op1=mybir.AluOpType.mult,
                )
                nc.sync.dma_start(outd[t], ob[:])

    nc.compile()
    return nc


_CACHED_NC = None


def kernel(h_v, h_w, e_vw, W1, b1, W2, b2, W3, b3, W4, b4):
    global LAST_RESULTS, _CACHED_NC

    h_w = np.asarray(h_w, np.float32)
    e_vw = np.asarray(e_vw, np.float32)
    W1 = np.asarray(W1, np.float32)
    W2 = np.asarray(W2, np.float32)
    W3 = np.asarray(W3, np.float32)
    W4 = np.asarray(W4, np.float32)
    b1 = np.asarray(b1, np.float32)
    b2 = np.asarray(b2, np.float32)
    b3 = np.asarray(b3, np.float32)
    b4 = np.asarray(b4, np.float32)

    # Host-side weight transform (exact reassociation of the reference math).
    W4s = W4.reshape(HID3, OUT_F, IN_F).sum(axis=2)
    b4s = b4.reshape(OUT_F, IN_F).sum(axis=1)
    s = h_w.reshape(-1)

    w3p = np.concatenate([W3[0:128], W3[128:256]], axis=1)  # [128, 256]
    bb = np.zeros((128, 5), np.float32)
    bb[:, 0] = b1
    bb[:, 1] = b2[0:128]
    bb[:, 2] = b2[128:256]
    bb[:, 3] = b3
    bb[0:OUT_F, 4] = b4s

    weights_map = {
        "w1d": np.ascontiguousarray(W1, NP_DT),
        "w2d": np.ascontiguousarray(W2, NP_DT),
        "w3d": np.ascontiguousarray(w3p, NP_DT),
        "w4d": np.ascontiguousarray(W4s, NP_DT),
        "bbd": bb,
    }

    in_maps = []
    for c in range(N_CORES):
        sl = slice(c * E_LOC, (c + 1) * E_LOC)
        e_loc = e_vw[sl]                       # [4096, 32]
        s_loc = s[sl]                          # [4096]
        e_t = np.ascontiguousarray(e_loc.T, NP_DT)   # [32, 4096]
        s_tiles = np.empty((NT, OUT_F, TILE), np.float32)
        for t in range(NT):
            s_tiles[t] = s_loc[t * TILE : (t + 1) * TILE][None, :]
        in_maps.append({"e_t": e_t, "s_b": s_tiles, **weights_map})

    if _CACHED_NC is None:
        _CACHED_NC = _build_bass()
    nc = _CACHED_NC

    trace = bool(int(os.environ.get("KERNEL_TRACE", "0")))
    res = run_bass_kernel_spmd(
        nc, in_maps, core_ids=list(range(N_CORES)), trace=trace
    )
    LAST_RESULTS = res

    out = np.empty((E, OUT_F), np.float32)
    for c in range(N_CORES):
        o = res.results[c]["outd"]             # [NT, OUT_F, TILE]
        base = c * E_LOC
        for t in range(NT):
            out[base + t * TILE : base + (t + 1) * TILE] = o[t].T
    return out


# revision 17
# speedup vs baseline: 1.2571x; 1.2571x over previous
"""Trainium2 Bass kernel for nn_MessageFunction (gnn_message_passing).

Math (validated against the reference):
  The reference broadcasts h_w[:, :, None] -> (B*N, IN_F, N) and reshapes to
  [E, IN_F]; row-major order makes every row constant:
      h_w_rows[e, i] = h_w.reshape(-1)[e]   for all i.
  Hence the per-edge bmm collapses:
      m[e, o] = sum_i edge_output[e, o, i] * s[e]
              = s[e] * (x3[e] @ W4s[:, o] + b4s[o])
  with W4s = W4.reshape(HID3, OUT_F, IN_F).sum(-1), b4s = b4.reshape(OUT_F,
  IN_F).sum(-1), s = h_w.reshape(-1).  This is an exact reassociation (only
  f32 rounding differences) and removes the [E,128]@[128,4096] matmul + bmm.

Kernel: data-parallel over E = 32768 edges, 4096 per core across 8 cores,
MLP weights replicated, no cross-core communication.  Per core the MLP runs
features-on-partitions with edges streaming on the free dim:
    x1 = relu(W1.T @ eT)        K=32  -> [128, e]
    x2 = relu(W2.T @ x1)        K=128 -> [256, e] (two 128-part halves)
    x3 = relu(W3.T @ x2)        K=256 -> [128, e] (PSUM accumulation)
    y  = W4s.T @ x3             K=128 -> [64, e]  (col-packed 2 tiles/PSUM)
    out = (y + b4s) * s         one fused scalar_tensor_tensor on VectorE
Matmuls use float32r (full PE rate at N=512, near-fp32 precision).
"""

import os

import numpy as np

import concourse.bacc as bacc
import concourse.bass as bass
import concourse.mybir as mybir
import concourse.tile as tile
from concourse.bass_utils import run_bass_kernel_spmd

# Problem constants (hardcoded per the harness contract).
B, N = 8, 64
IN_F, OUT_F = 64, 64
EDGE_F = 32
HID1, HID2, HID3 = 128, 256, 128
E = B * N * N            # 32768
N_CORES = 8
E_LOC = E // N_CORES     # 4096
TILE = 512               # edges per tile (one PSUM bank per stage)
NT = E_LOC // TILE       # 8 tiles per core
OUT_CHUNK = 1024         # output DMA granularity (2 tiles)

F32 = mybir.dt.float32
# Matmul operand dtype: float32r streams at 1 cycle/row for N>=256 (same as
# bf16) with much better precision than bf16.
DT = mybir.dt.float32r
NP_DT = np.float32

# Module global: last BassKernelResults (test.py reads exec_time_ns from it).
LAST_RESULTS = None


def _build_bass():
    nc = bacc.Bacc(
        "TRN2", target_bir_lowering=False, debug=False, num_devices=N_CORES
    )

    # Per-core inputs
    e_t = nc.dram_tensor("e_t", [EDGE_F, E_LOC], DT, kind="ExternalInput")
    s_b = nc.dram_tensor("s_b", [OUT_F, E_LOC], F32, kind="ExternalInput")
    # Replicated weights
    w1d = nc.dram_tensor("w1d", [EDGE_F, HID1], DT, kind="ExternalInput")
    w2d = nc.dram_tensor("w2d", [HID1, HID2], DT, kind="ExternalInput")
    # W3 packed side by side: [:, 0:128] = W3[0:128, :], [:, 128:256] = W3[128:256, :]
    w3d = nc.dram_tensor("w3d", [128, 2 * HID3], DT, kind="ExternalInput")
    w4d = nc.dram_tensor("w4d", [HID3, OUT_F], DT, kind="ExternalInput")
    # Bias columns: b1, b2[:128], b2[128:], b3, [b4s; pad]
    bbd = nc.dram_tensor("bbd", [128, 5], F32, kind="ExternalInput")
    outd = nc.dram_tensor(
        "outd", [E_LOC // OUT_CHUNK, OUT_F, OUT_CHUNK], F32, kind="ExternalOutput"
    )

    # Relu pass engine schedule (per tile: L1, L2a, L2b, L3). 'A' = ScalarE,
    # 'V' = VectorE.  VectorE also runs the eight final bias+scale ops, so
    # ScalarE takes more of the 32 relu passes (20 A / 12 V).
    relu_sched = ["AVAV", "AVAA"] * (NT // 2)

    with tile.TileContext(nc) as tc:
        with (
            tc.tile_pool(name="wp", bufs=1) as wp,
            tc.tile_pool(name="io", bufs=4) as io,
            tc.tile_pool(name="acts", bufs=3) as acts,
            tc.tile_pool(name="ps", bufs=1, space="PSUM") as ps,
        ):
            w1 = wp.tile([EDGE_F, HID1], DT, tag="w1")
            w2 = wp.tile([HID1, HID2], DT, tag="w2")
            w3 = wp.tile([128, 2 * HID3], DT, tag="w3")
            w4 = wp.tile([HID3, OUT_F], DT, tag="w4")
            bb = wp.tile([128, 5], F32, tag="bb")
            s_sb = wp.tile([OUT_F, E_LOC], F32, tag="s_sb")
            out_sb = wp.tile([OUT_F, E_LOC], F32, tag="out_sb")
            nc.sync.dma_start(w1[:], w1d[:])
            nc.sync.dma_start(w2[:], w2d[:])
            nc.sync.dma_start(w3[:], w3d[:])
            nc.sync.dma_start(w4[:], w4d[:])
            nc.sync.dma_start(bb[:], bbd[:])
            nc.sync.dma_start(s_sb[:], s_b[:])

            def relu_pass(dst, src, bias_col, eng):
                if eng == "A":
                    nc.scalar.activation(
                        dst, src, mybir.ActivationFunctionType.Relu, bias=bias_col
                    )
                else:
                    nc.vector.tensor_scalar(
                        out=dst,
                        in0=src,
                        scalar1=bias_col,
                        scalar2=0.0,
                        op0=mybir.AluOpType.add,
                        op1=mybir.AluOpType.max,
                    )

            for t in range(NT):
                sched = relu_sched[t]
                cs = slice(t * TILE, (t + 1) * TILE)

                et = io.tile([EDGE_F, TILE], DT, tag="et")
                nc.sync.dma_start(et[:], e_t[:, cs])

                # L1: [32,128].T @ [32,512]
                x1p = ps.tile([128, TILE], F32, tag="x1p", bufs=2)
                nc.tensor.matmul(x1p[:], w1[:], et[:])
                x1 = acts.tile([128, TILE], DT, tag="x1")
                relu_pass(x1[:], x1p[:], bb[:, 0:1], sched[0])

                # L2: [128,256] -> two 128-part halves
                x2ap = ps.tile([128, TILE], F32, tag="x2ap", bufs=1)
                x2bp = ps.tile([128, TILE], F32, tag="x2bp", bufs=1)
                nc.tensor.matmul(x2ap[:], w2[:, 0:128], x1[:])
                nc.tensor.matmul(x2bp[:], w2[:, 128:256], x1[:])
                x2a = acts.tile([128, TILE], DT, tag="x2a")
                x2b = acts.tile([128, TILE], DT, tag="x2b")
                relu_pass(x2a[:], x2ap[:], bb[:, 1:2], sched[1])
                relu_pass(x2b[:], x2bp[:], bb[:, 2:3], sched[2])

                # L3: K=256 via PSUM accumulation of two K=128 matmuls
                x3ps = ps.tile([128, TILE], F32, tag="x3ps", bufs=2)
                nc.tensor.matmul(x3ps[:], w3[:, 0:128], x2a[:], start=True, stop=False)
                nc.tensor.matmul(x3ps[:], w3[:, 128:256], x2b[:], start=False, stop=True)
                x3 = acts.tile([128, TILE], DT, tag="x3")
                relu_pass(x3[:], x3ps[:], bb[:, 3:4], sched[3])

                # L4: [128,64].T @ [128,512] -> [64, 512]
                y4p = ps.tile([OUT_F, TILE], F32, tag="y4p", bufs=2)
                nc.tensor.matmul(y4p[:], w4[:], x3[:])
                # out = (y4 + b4s) * s   (single fused VectorE op)
                nc.vector.scalar_tensor_tensor(
                    out=out_sb[:, cs],
                    in0=y4p[:],
                    scalar=bb[0:OUT_F, 4:5],
                    in1=s_sb[:, cs],
                    op0=mybir.AluOpType.add,
                    op1=mybir.AluOpType.mult,
                )
                if (t + 1) * TILE % OUT_CHUNK == 0:
                    ck = ((t + 1) * TILE) // OUT_CHUNK - 1
                    nc.sync.dma_start(
                        outd[ck],
                        out_sb[:, ck * OUT_CHUNK : (ck + 1) * OUT_CHUNK],
                    )

    nc.compile()
    return nc


_CACHED_NC = None


def kernel(h_v, h_w, e_vw, W1, b1, W2, b2, W3, b3, W4, b4):
    global LAST_RESULTS, _CACHED_NC

    h_w = np.asarray(h_w, np.float32)
    e_vw = np.asarray(e_vw, np.float32)
    W1 = np.asarray(W1, np.float32)
    W2 = np.asarray(W2, np.float32)
    W3 = np.asarray(W3, np.float32)
    W4 = np.asarray(W4, np.float32)
    b1 = np.asarray(b1, np.float32)
    b2 = np.asarray(b2, np.float32)
    b3 = np.asarray(b3, np.float32)
    b4 = np.asarray(b4, np.float32)

    # Host-side weight transform (exact reassociation of the reference math).
    W4s = W4.reshape(HID3, OUT_F, IN_F).sum(axis=2)
    b4s = b4.reshape(OUT_F, IN_F).sum(axis=1)
    s = h_w.reshape(-1)

    w3p = np.concatenate([W3[0:128], W3[128:256]], axis=1)  # [128, 256]
    bb = np.zeros((128, 5), np.float32)
    bb[:, 0] = b1
    bb[:, 1] = b2[0:128]
    bb[:, 2] = b2[128:256]
    bb[:, 3] = b3
    bb[0:OUT_F, 4] = b4s

    weights_map = {
        "w1d": np.ascontiguousarray(W1, NP_DT),
        "w2d": np.ascontiguousarray(W2, NP_DT),
        "w3d": np.ascontiguousarray(w3p, NP_DT),
        "w4d": np.ascontiguousarray(W4s, NP_DT),
        "bbd": bb,
    }

    in_maps = []
    for c in range(N_CORES):
        sl = slice(c * E_LOC, (c + 1) * E_LOC)
        e_loc = e_vw[sl]                       # [4096, 32]
        s_loc = s[sl]                          # [4096]
        e_t = np.ascontiguousarray(e_loc.T, NP_DT)   # [32, 4096]
        s_bcast = np.ascontiguousarray(
            np.broadcast_to(s_loc[None, :], (OUT_F, E_LOC)), np.float32
        )
        in_maps.append({"e_t": e_t, "s_b": s_bcast, **weights_map})

    if _CACHED_NC is None:
        _CACHED_NC = _build_bass()
    nc = _CACHED_NC

    trace = bool(int(os.environ.get("KERNEL_TRACE", "0")))
    res = run_bass_kernel_spmd(
        nc, in_maps, core_ids=list(range(N_CORES)), trace=trace
    )
    LAST_RESULTS = res

    out = np.empty((E, OUT_F), np.float32)
    nck = E_LOC // OUT_CHUNK
    for c in range(N_CORES):
        o = res.results[c]["outd"]             # [nck, OUT_F, OUT_CHUNK]
        base = c * E_LOC
        for k in range(nck):
            out[base + k * OUT_CHUNK : base + (k + 1) * OUT_CHUNK] = o[k].T
    return out
